# revision 1
# baseline (speedup 1.0000x reference)
"""Multi-head attention (B=2, S=2048, D=1024, H=16, dk=64) on 8 TRN2 cores.

Sharding: core c -> (batch b = c//4, head-group g = c%4 of 4 heads).
Each core computes q/k/v projections for its 4 heads, full attention for
those heads, and a partial output projection (rows g*256:(g+1)*256 of Wo).
Host pre-transposes/casts inputs to bf16 and sums the partial outputs.

Device layout (per core, all matmul operands bf16, accumulation f32):
  xqT/xkT/xvT [1024, 2048]   (d on partitions -> contraction-ready)
  qT, kT      [256, 2048]    (head-dim on partitions; pair tiles [128, S])
  v_aug       [2048, 4*65]   (per head: [v_h | ones]; ones col => softmax denom)
  scoresT     [j, i] in PSUM; exp on ScalarE -> probsT bf16 (no max-subtract:
              scores ~ N(0,1) after 1/8 scaling, exp bounded ~e^6)
  PV:         attnT_unnorm[e, i] = sum_j v_aug[j, e] * probsT[j, i]
              (row 64 = softmax denominator), normalize via reciprocal +
              K=1 broadcast matmul, store attnT [64, S] per head
  out-projT:  outT[n, s] = sum_{h,e} wo[h,e,n] * attnT_h[e, s]  (K=64 x4)
Host: out[b] = sum_g outT_partial.T + (bv @ Wo + bo).
"""

import os

import numpy as np
import ml_dtypes

BF16 = ml_dtypes.bfloat16

B, S, D = 2, 2048, 1024
H, DK = 16, 64
P = 128
GROUPS = 4          # head groups (one per core within a batch)
HPG = 4             # heads per group
GD = HPG * DK       # 256, group width
KC = D // P         # 8 contraction chunks
ST = S // P         # 16 s-tiles / j-tiles
NCORES = 8
FP8_PV = False      # fp8 PV measured 3.7e-2 rel err (e4m3 noise) - keep bf16
DEBUG_DUMP = False  # extra outputs: per-head attnT and denominators

_cached = {}


def _build_bass():
    import concourse.bass as bass
    import concourse.tile as tile
    from concourse.bacc import Bacc
    from concourse import mybir
    from contextlib import ExitStack

    f32 = mybir.dt.float32
    bf16 = mybir.dt.bfloat16
    Act = mybir.ActivationFunctionType

    nc = Bacc()

    xqT = nc.dram_tensor("xqT", [D, S], bf16, kind="ExternalInput")
    xkT = nc.dram_tensor("xkT", [D, S], bf16, kind="ExternalInput")
    xvT = nc.dram_tensor("xvT", [D, S], bf16, kind="ExternalInput")
    wq = nc.dram_tensor("wq", [D, GD], bf16, kind="ExternalInput")
    wk = nc.dram_tensor("wk", [D, GD], bf16, kind="ExternalInput")
    wv = nc.dram_tensor("wv", [D, GD], bf16, kind="ExternalInput")
    wo = nc.dram_tensor("wo", [GD, D], bf16, kind="ExternalInput")
    bq = nc.dram_tensor("bq", [GD, 1], f32, kind="ExternalInput")
    bk = nc.dram_tensor("bk", [GD, 1], f32, kind="ExternalInput")
    out = nc.dram_tensor("out", [S, D], f32, kind="ExternalOutput")

    with tile.TileContext(nc) as tc, ExitStack() as ctx:
        singles = ctx.enter_context(tc.tile_pool(name="singles", bufs=1))
        probs_pool = ctx.enter_context(tc.tile_pool(name="probs", bufs=3))
        small = ctx.enter_context(tc.tile_pool(name="small", bufs=8))
        outs_pool = ctx.enter_context(tc.tile_pool(name="outs", bufs=8))
        psum = ctx.enter_context(tc.tile_pool(name="psum", bufs=1, space="PSUM"))

        # ---- persistent SBUF ----
        wq_sb = singles.tile([P, KC, GD], bf16)
        wk_sb = singles.tile([P, KC, GD], bf16)
        wv_sb = singles.tile([P, KC, GD], bf16)
        wo_sb = singles.tile([P, 2, D], bf16)
        bq_sb = singles.tile([P, 2, 1], f32)
        bk_sb = singles.tile([P, 2, 1], f32)
        nc.sync.dma_start(out=wq_sb, in_=wq.rearrange("(c p) m -> p c m", p=P))
        nc.sync.dma_start(out=wk_sb, in_=wk.rearrange("(c p) m -> p c m", p=P))
        nc.sync.dma_start(out=wv_sb, in_=wv.rearrange("(c p) m -> p c m", p=P))
        nc.sync.dma_start(out=wo_sb, in_=wo.rearrange("(c p) n -> p c n", p=P))
        nc.sync.dma_start(out=bq_sb, in_=bq.rearrange("(t p) o -> p t o", p=P))
        nc.sync.dma_start(out=bk_sb, in_=bk.rearrange("(t p) o -> p t o", p=P))

        xq_sb = singles.tile([P, KC, S], bf16)
        xk_sb = singles.tile([P, KC, S], bf16)
        xv_sb = singles.tile([P, KC, S], bf16)
        # tensor-by-tensor so q-proj can start after the first xq chunk
        # and PE chases the DMA stream instead of waiting on all three
        for k in range(KC):
            nc.sync.dma_start(out=xq_sb[:, k, :], in_=xqT[k * P:(k + 1) * P, :])
        for k in range(KC):
            nc.sync.dma_start(out=xk_sb[:, k, :], in_=xkT[k * P:(k + 1) * P, :])
        for k in range(KC):
            nc.sync.dma_start(out=xv_sb[:, k, :], in_=xvT[k * P:(k + 1) * P, :])

        qT_sb = [singles.tile([P, S], bf16, name=f"qT{t}") for t in range(2)]
        kT_sb = [singles.tile([P, S], bf16, name=f"kT{t}") for t in range(2)]
        # attnT per head pair [128 hd, S]: even head at partitions 0:64
        # (written directly by DVE), odd head at 64:128 (DVE writes a base-0
        # staging tile, then SBUF->SBUF DMA relocates partitions - engines
        # are lane-locked but DMA is not). Enables K=128 out-projection.
        att_pair = [singles.tile([P, S], bf16, name=f"attp{p}")
                    for p in range(2)]
        att_odd = [singles.tile([DK, S], bf16, name=f"atto{p}")
                   for p in range(2)]

        ones_sb = singles.tile([65, DK], f32)
        nc.vector.memset(ones_sb[64:65, :], 1.0)

        CP = ST // 2
        if FP8_PV:
            fp8 = mybir.dt.float8e4
            # [j-in-chunk, chunk-pair, chunk-in-pair, head, 64 v cols + 1 one + pad]
            v_sb = singles.tile([P, CP, 2, HPG, 80], fp8)
            nc.vector.memset(v_sb[:, :, :, :, 64:65], 1.0)
            v4 = None
            # exp(s/8 - 3): keeps exp within IEEE e4m3 range (max finite 240;
            # max observed score ~7.7 -> e^4.7 ~ 110). Softmax shift-invariant.
            exp_bias = singles.tile([P, 1], f32)
            nc.vector.memset(exp_bias, -3.0)
        else:
            v_sb = singles.tile([P, ST, HPG * 65], bf16)
            # ones columns of v_aug (col 64 of each per-head [64|1] block)
            v4 = v_sb.rearrange("p s (h c) -> p s h c", c=65)
            nc.vector.memset(v4[:, :, :, 64:65], 1.0)

        # ---- phase A: projections ----
        def qk_proj(x_sb, w_sb, b_sb, dst, t):
            pq = [psum.tile([P, 1024], mybir.dt.float32, tag="sc", bufs=2,
                            name=f"pq{t}{half}") for half in range(2)]
            for k in range(KC):
                for half in range(2):
                    for sq in range(2):
                        nc.tensor.matmul(
                            out=pq[half][:, sq * 512:(sq + 1) * 512],
                            lhsT=w_sb[:, k, t * P:(t + 1) * P],
                            rhs=x_sb[:, k, half * 1024 + sq * 512:
                                     half * 1024 + (sq + 1) * 512],
                            start=(k == 0), stop=(k == KC - 1))
            for half in range(2):
                nc.vector.tensor_scalar_add(
                    out=dst[:, half * 1024:(half + 1) * 1024],
                    in0=pq[half], scalar1=b_sb[:, t, :])

        def v_proj():
            for st in range(ST):
                pvv = psum.tile([P, GD], mybir.dt.float32, tag="pv", bufs=4, name="pvv")
                for k in range(KC):
                    nc.tensor.matmul(
                        out=pvv,
                        lhsT=xv_sb[:, k, st * P:(st + 1) * P],
                        rhs=wv_sb[:, k, :],
                        start=(k == 0), stop=(k == KC - 1))
                if FP8_PV:
                    dst = v_sb[:, st // 2, st % 2, :, 0:64]
                else:
                    dst = v4[:, st, :, 0:64]
                src = pvv.rearrange("p (h c) -> p h c", c=64)
                nc.vector.tensor_copy(out=dst, in_=src)

        # ---- phase B: attention for one head pair, one i-half ----
        # `pending` = previous iteration's normalize emitter; it is emitted
        # after this iteration's first two j-tiles so ACT/PE stay fed across
        # the (pair, ih) boundary. Returns this iteration's normalize.
        def attention(pair, ih, pending=None):
            pv = [[psum.tile([65, 512], mybir.dt.float32, tag="pv", bufs=4,
                             name=f"pv{pair}{ih}{hp}{iq}")
                   for iq in range(2)] for hp in range(2)]
            if FP8_PV:
                fp8 = mybir.dt.float8e4
                for cp in range(CP):
                    pr = [probs_pool.tile([P, 2, 1024], fp8, tag="probs",
                                          name=f"pr{hp}") for hp in range(2)]
                    for d in range(2):
                        jt = 2 * cp + d
                        sc = [psum.tile([P, 1024], mybir.dt.float32, tag="sc",
                                        bufs=2, name=f"sc{hp}")
                              for hp in range(2)]
                        for iq in range(2):
                            for hp in range(2):
                                nc.tensor.matmul(
                                    out=sc[hp][:, iq * 512:(iq + 1) * 512],
                                    lhsT=kT_sb[pair][hp * 64:(hp + 1) * 64,
                                                     jt * P:(jt + 1) * P],
                                    rhs=qT_sb[pair][hp * 64:(hp + 1) * 64,
                                                    ih * 1024 + iq * 512:
                                                    ih * 1024 + (iq + 1) * 512],
                                    start=True, stop=True)
                        for hp in range(2):
                            # exp(s/8 - 2): global shift keeps exp within
                            # e4m3 range (softmax is shift-invariant)
                            nc.scalar.activation(out=pr[hp][:, d, :],
                                                 in_=sc[hp], func=Act.Exp,
                                                 scale=0.125, bias=exp_bias)
                    for hp in range(2):
                        h = 2 * pair + hp
                        for iq in range(2):
                            nc.tensor.matmul(
                                out=pv[hp][iq][:, :],
                                lhsT=v_sb[:, cp, :, h, 0:65],
                                rhs=pr[hp][:, :, iq * 512:(iq + 1) * 512],
                                perf_mode=mybir.MatmulPerfMode.DoubleRow,
                                start=(cp == 0), stop=(cp == CP - 1))
                    if cp == 1 and pending is not None:
                        pending()
            else:
                for jt in range(ST):
                    sc = [psum.tile([P, 1024], mybir.dt.float32, tag="sc",
                                    bufs=2, name=f"sc{hp}") for hp in range(2)]
                    for iq in range(2):
                        for hp in range(2):
                            nc.tensor.matmul(
                                out=sc[hp][:, iq * 512:(iq + 1) * 512],
                                lhsT=kT_sb[pair][hp * 64:(hp + 1) * 64,
                                                 jt * P:(jt + 1) * P],
                                rhs=qT_sb[pair][hp * 64:(hp + 1) * 64,
                                                ih * 1024 + iq * 512:
                                                ih * 1024 + (iq + 1) * 512],
                                start=True, stop=True)
                    for hp in range(2):
                        probs = probs_pool.tile([P, 1024], bf16, tag="probs",
                                                name="probs")
                        nc.scalar.activation(out=probs, in_=sc[hp],
                                             func=Act.Exp, scale=0.125)
                        h65 = (2 * pair + hp) * 65
                        for iq in range(2):
                            nc.tensor.matmul(
                                out=pv[hp][iq][:, :],
                                lhsT=v_sb[:, jt, h65:h65 + 65],
                                rhs=probs[:, iq * 512:(iq + 1) * 512],
                                start=(jt == 0), stop=(jt == ST - 1))
                    if jt == 1 and pending is not None:
                        pending()

            def normalize():
                for hp in range(2):
                    for iq in range(2):
                        r = small.tile([65, 512], mybir.dt.float32, tag="r",
                                       name="r")
                        nc.vector.reciprocal(out=r[64:65, :],
                                             in_=pv[hp][iq][64:65, :])
                        bc = psum.tile([64, 512], mybir.dt.float32, tag="pv",
                                       bufs=4, name="bc")
                        nc.tensor.matmul(out=bc, lhsT=ones_sb[64:65, :],
                                         rhs=r[64:65, :], start=True,
                                         stop=True)
                        pvs = small.tile([64, 512], mybir.dt.float32,
                                         tag="pvs", name="pvs")
                        nc.vector.tensor_copy(out=pvs, in_=pv[hp][iq][0:64, :])
                        col = ih * 1024 + iq * 512
                        if hp == 0:
                            nc.vector.tensor_mul(
                                out=att_pair[pair][0:64, col:col + 512],
                                in0=pvs, in1=bc)
                        else:
                            nc.vector.tensor_mul(
                                out=att_odd[pair][:, col:col + 512],
                                in0=pvs, in1=bc)
                            nc.sync.dma_start(
                                out=att_pair[pair][64:128, col:col + 512],
                                in_=att_odd[pair][:, col:col + 512])

            return normalize

        def out_proj():
            # out[s, n] = sum_c att_pair[c].T @ wo_chunk[c]  (K=128 per chunk)
            for st in range(ST):
                po = [psum.tile([P, 512], mybir.dt.float32, tag="pv", bufs=4,
                                name=f"po{nb}") for nb in range(2)]
                for c in range(2):
                    for nb in range(2):
                        nc.tensor.matmul(
                            out=po[nb],
                            lhsT=att_pair[c][:, st * P:(st + 1) * P],
                            rhs=wo_sb[:, c, nb * 512:(nb + 1) * 512],
                            start=(c == 0), stop=(c == 1))
                for nb in range(2):
                    osb = outs_pool.tile([P, 512], mybir.dt.float32,
                                         tag="osb", name="osb")
                    if nb % 2 == 0:
                        nc.vector.tensor_copy(out=osb, in_=po[nb])
                    else:
                        nc.scalar.copy(out=osb, in_=po[nb])
                    nc.sync.dma_start(
                        out=out[st * P:(st + 1) * P,
                                nb * 512:(nb + 1) * 512],
                        in_=osb)

        qk_proj(xq_sb, wq_sb, bq_sb, qT_sb[0], 0)
        qk_proj(xk_sb, wk_sb, bk_sb, kT_sb[0], 0)
        v_proj()
        qk_proj(xq_sb, wq_sb, bq_sb, qT_sb[1], 1)
        qk_proj(xk_sb, wk_sb, bk_sb, kT_sb[1], 1)
        pending = None
        for pair in range(2):
            for ih in range(2):
                pending = attention(pair, ih, pending)
        pending()
        out_proj()

    nc.finalize()
    return nc


def kernel(Q, K, V, Wq, bq, Wk, bk, Wv, bv, Wo, bo):
    from concourse.bass_utils import run_bass_kernel_spmd

    f32 = np.float32
    Q = np.asarray(Q, f32)
    K = np.asarray(K, f32)
    V = np.asarray(V, f32)
    Wq = np.asarray(Wq, f32)
    Wk = np.asarray(Wk, f32)
    Wv = np.asarray(Wv, f32)
    Wo = np.asarray(Wo, f32)
    bq = np.asarray(bq, f32)
    bk = np.asarray(bk, f32)
    bv = np.asarray(bv, f32)
    bo = np.asarray(bo, f32)

    xT = {}
    for b in range(B):
        xT[('q', b)] = np.ascontiguousarray(Q[b].T).astype(BF16)
        xT[('k', b)] = np.ascontiguousarray(K[b].T).astype(BF16)
        xT[('v', b)] = np.ascontiguousarray(V[b].T).astype(BF16)

    in_maps = []
    for c in range(NCORES):
        b, g = c // GROUPS, c % GROUPS
        sl = slice(g * GD, (g + 1) * GD)
        in_maps.append({
            "xqT": xT[('q', b)],
            "xkT": xT[('k', b)],
            "xvT": xT[('v', b)],
            "wq": np.ascontiguousarray(Wq[:, sl]).astype(BF16),
            "wk": np.ascontiguousarray(Wk[:, sl]).astype(BF16),
            "wv": np.ascontiguousarray(Wv[:, sl]).astype(BF16),
            "wo": np.ascontiguousarray(Wo[sl, :]).astype(BF16),
            "bq": np.ascontiguousarray(bq[sl].reshape(GD, 1)),
            "bk": np.ascontiguousarray(bk[sl].reshape(GD, 1)),
        })

    if "nc" not in _cached:
        _cached["nc"] = _build_bass()
    nc = _cached["nc"]

    try:
        res = run_bass_kernel_spmd(nc, in_maps, core_ids=list(range(NCORES)))
    except ModuleNotFoundError:
        # BASS_TRACE set but the axon ntff hook isn't shipped in this
        # container - retry untraced
        os.environ["BASS_NEVER_TRACE"] = "1"
        res = run_bass_kernel_spmd(nc, in_maps, core_ids=list(range(NCORES)))
    if res.exec_time_ns is not None:
        print(f"HW exec time: {res.exec_time_ns} ns")

    bo_eff = (bv @ Wo + bo).astype(f32)
    out = np.zeros((B, S, D), f32)
    for c in range(NCORES):
        b = c // GROUPS
        out[b] += res.results[c]["out"]
    out += bo_eff
    return out



# revision 21
# speedup vs baseline: 1.2519x; 1.2519x over previous
"""Multi-head attention (B=2, S=2048, D=1024, H=16, dk=64) on 8 TRN2 cores.

Sharding: core c -> (batch b = c//4, head-group g = c%4 of 4 heads).

Design (v2, ACT-paced pipeline):
  The softmax exp is the hard floor: 4 heads x 2048^2 scores / 128 lanes
  on the one engine that can exp (ACT) ~= 133us.  Everything else is
  scheduled to hide under it:
  - inputs stream in 256-column s-slices (full-bandwidth 4KB runs) so the
    first exp fires at ~11us instead of ~26us;
  - scoresT[j,i] -> exp -> probs[j,i] (bf16, SBUF);
  - PV in [i,e] orientation: out[i,64] = probs[j,i-chunk]^T @ v[j,64],
    F=64 accumulating matmuls (half the PE rows of the [e,i] form);
  - softmax denominators recomputed at each head-pair boundary from the
    buffered probs tiles (F=1 accumulating matmuls into a transient PSUM
    tile), reciprocal + per-partition scalar-mul normalize on DVE;
  - normalized attn [i,(hp,e)] transposed per 128-chunk on the PE
    (one [128,128] transpose covers both heads), giving attT[e,s] for a
    K=128x2 output projection;
  - out projection for the first s-half runs inside the second half's
    attention; bf16 partial outputs (host reduces in f32).
  PSUM: scores 2x[128,1024](4 banks) + pvacc 2x[128,512](2) + work ring
  2x[128,512](2) = 8 banks exactly.
"""

import os
from collections import deque

import numpy as np
import ml_dtypes

BF16 = ml_dtypes.bfloat16

B, S, D = 2, 2048, 1024
H, DK = 16, 64
P = 128
GROUPS = 4
HPG = 4             # heads per group (2 pairs)
GD = HPG * DK       # 256
KC = D // P         # 8 contraction chunks
NSL = 8             # 256-wide s-slices per tensor
SLW = S // NSL      # 256
ST = S // P         # 16 j-tiles / s-tiles
NCORES = 8
PV_LAG = 5          # steps between exp(jt) and pv(jt)

_cached = {}
DEBUG = False


def _build_bass():
    import concourse.bass as bass
    import concourse.tile as tile
    from concourse.bacc import Bacc
    from concourse import mybir
    from contextlib import ExitStack

    f32 = mybir.dt.float32
    bf16 = mybir.dt.bfloat16
    Act = mybir.ActivationFunctionType

    nc = Bacc()

    xq = nc.dram_tensor("xq", [P, NSL * KC * SLW], bf16, kind="ExternalInput")
    xk = nc.dram_tensor("xk", [P, NSL * KC * SLW], bf16, kind="ExternalInput")
    xv = nc.dram_tensor("xv", [P, NSL * KC * SLW], bf16, kind="ExternalInput")
    wq = nc.dram_tensor("wq", [P, KC * GD], bf16, kind="ExternalInput")
    wk = nc.dram_tensor("wk", [P, KC * GD], bf16, kind="ExternalInput")
    wv = nc.dram_tensor("wv", [P, KC * GD], bf16, kind="ExternalInput")
    wo = nc.dram_tensor("wo", [P, 2 * D], bf16, kind="ExternalInput")
    bq = nc.dram_tensor("bq", [P, 2], f32, kind="ExternalInput")
    bk = nc.dram_tensor("bk", [P, 2], f32, kind="ExternalInput")
    ident = nc.dram_tensor("ident", [P, P], bf16, kind="ExternalInput")
    out = nc.dram_tensor("out", [S, D], bf16, kind="ExternalOutput")
    if DEBUG:
        dbg = {
            'qT0': nc.dram_tensor("d_qT0", [P, S], bf16, kind="ExternalOutput"),
            'kT0': nc.dram_tensor("d_kT0", [P, S], bf16, kind="ExternalOutput"),
            'v': nc.dram_tensor("d_v", [P, ST * HPG * DK], bf16,
                                kind="ExternalOutput"),
            'pr000': nc.dram_tensor("d_pr000", [P, 1024], bf16,
                                    kind="ExternalOutput"),
            'rec00': nc.dram_tensor("d_rec00", [P, 16], f32,
                                    kind="ExternalOutput"),
            'pv00': nc.dram_tensor("d_pv00", [P, 2 * 8 * DK], f32,
                                   kind="ExternalOutput"),
            'attT0': nc.dram_tensor("d_attT0", [P, S], bf16,
                                    kind="ExternalOutput"),
            'attT1': nc.dram_tensor("d_attT1", [P, S], bf16,
                                    kind="ExternalOutput"),
        }

    with tile.TileContext(nc) as tc, ExitStack() as ctx:
        singles = ctx.enter_context(tc.tile_pool(name="singles", bufs=1))
        xring = ctx.enter_context(tc.tile_pool(name="xring", bufs=4))
        probs_pool = ctx.enter_context(tc.tile_pool(name="probs", bufs=44))
        attn_pool = ctx.enter_context(tc.tile_pool(name="attn", bufs=8))
        rec_pool = ctx.enter_context(tc.tile_pool(name="rec", bufs=2))
        outs_pool = ctx.enter_context(tc.tile_pool(name="outs", bufs=3))
        psum = ctx.enter_context(tc.tile_pool(name="psum", bufs=1, space="PSUM"))

        # ---------------- persistent SBUF ----------------
        wq_sb = singles.tile([P, KC, GD], bf16)
        wk_sb = singles.tile([P, KC, GD], bf16)
        wv_sb = singles.tile([P, KC, GD], bf16)
        wo_sb = singles.tile([P, 2, D], bf16)
        bq_sb = singles.tile([P, 2], f32)
        bk_sb = singles.tile([P, 2], f32)
        ident_sb = singles.tile([P, P], bf16)
        ones_sb = singles.tile([P, 1], bf16)
        dummy_sb = singles.tile([P, 1], f32)

        qT_sb = [singles.tile([P, S], bf16, name=f"qT{t}") for t in range(2)]
        kT_sb = [singles.tile([P, S], bf16, name=f"kT{t}") for t in range(2)]
        v_sb = singles.tile([P, ST, HPG, DK], bf16)
        attT = [singles.tile([P, S], bf16, name=f"attT{p}") for p in range(2)]

        # warm the ACT exp table during the DMA lead-in
        nc.vector.memset(ones_sb, 1.0)
        nc.vector.memset(dummy_sb, 0.0)
        warm = singles.tile([P, 1], f32)
        nc.scalar.activation(out=warm, in_=dummy_sb, func=Act.Exp)

        # ---------------- DMA stream (order = schedule) ----------------
        nc.sync.dma_start(out=ident_sb, in_=ident[:, :])
        nc.sync.dma_start(out=bq_sb, in_=bq[:, :])
        nc.sync.dma_start(out=bk_sb, in_=bk[:, :])
        nc.sync.dma_start(out=wq_sb, in_=wq.rearrange("p (c m) -> p c m", c=KC))
        nc.sync.dma_start(out=wk_sb, in_=wk.rearrange("p (c m) -> p c m", c=KC))

        # x slice rings: slot = [P, KC, SLW] bf16 (4KB/partition)
        x_slots = {}

        def dma_x(which, dram, sl):
            t = xring.tile([P, KC, SLW], bf16, tag=f"x{which}",
                           bufs=5 if which == 'k' else 4,
                           name=f"x{which}{sl}")
            nc.sync.dma_start(
                out=t, in_=dram.rearrange("p (sl c s) -> p sl c s",
                                          sl=NSL, c=KC)[:, sl])
            x_slots[(which, sl)] = t

        for sl in range(4):
            dma_x('q', xq, sl)
        dma_x('k', xk, 0)
        nc.sync.dma_start(out=wv_sb, in_=wv.rearrange("p (c m) -> p c m", c=KC))
        for g in range(1, 8):
            dma_x('k', xk, g)
            dma_x('v', xv, g - 1)
        dma_x('v', xv, 7)
        for sl in range(4, 8):
            dma_x('q', xq, sl)
        nc.sync.dma_start(out=wo_sb, in_=wo.rearrange("p (c n) -> p c n", c=2))

        # ---------------- unit emitters (memoized) ----------------
        emitted = set()
        vclock = [0.0]     # virtual PE ns

        def mm_cost(rows, n=1):
            return rows * 0.42 + n * 5.0

        def qkproj(which, t, sl):
            key = (which, t, sl)
            if key in emitted:
                return
            emitted.add(key)
            x_t = x_slots[(which, sl)]
            w_sb = wq_sb if which == 'q' else wk_sb
            b_sb = bq_sb if which == 'q' else bk_sb
            dstT = (qT_sb if which == 'q' else kT_sb)[t]
            pp = psum.tile([P, 512], f32, tag="work", bufs=2, name="pp")
            for c in range(KC):
                nc.tensor.matmul(out=pp[:, 0:SLW],
                                 lhsT=w_sb[:, c, t * P:(t + 1) * P],
                                 rhs=x_t[:, c, :],
                                 start=(c == 0), stop=(c == KC - 1))
            nc.vector.tensor_scalar_add(
                out=dstT[:, sl * SLW:(sl + 1) * SLW],
                in0=pp[:, 0:SLW], scalar1=b_sb[:, t:t + 1])
            vclock[0] += mm_cost(KC * SLW, KC)

        def vproj(jt):
            key = ('v', jt)
            if key in emitted:
                return
            emitted.add(key)
            sl, half = jt // 2, jt % 2
            x_t = x_slots[('v', sl)]
            pp = psum.tile([P, 512], f32, tag="work", bufs=2, name="pv")
            for c in range(KC):
                nc.tensor.matmul(
                    out=pp[:, 0:GD],
                    lhsT=x_t[:, c, half * P:(half + 1) * P],
                    rhs=wv_sb[:, c, :],
                    start=(c == 0), stop=(c == KC - 1))
            nc.vector.tensor_copy(
                out=v_sb[:, jt], in_=pp[:, 0:GD].rearrange("p (h e) -> p h e",
                                                           e=DK))
            vclock[0] += mm_cost(KC * GD, KC)

        probs_tiles = {}
        pvaccs = {}
        sc_seq = [0]

        def scores_exp(ih, pair, jt):
            qkproj('k', pair, jt // 2)
            for sl in range(ih * 4, ih * 4 + 4):
                qkproj('q', pair, sl)
            for hp in range(2):
                sc = psum.tile([P, 1024], f32, tag="sc", bufs=2,
                               name=f"sc{sc_seq[0] % 2}")
                sc_seq[0] += 1
                for iq in range(2):
                    nc.tensor.matmul(
                        out=sc[:, iq * 512:(iq + 1) * 512],
                        lhsT=kT_sb[pair][hp * DK:(hp + 1) * DK,
                                         jt * P:(jt + 1) * P],
                        rhs=qT_sb[pair][hp * DK:(hp + 1) * DK,
                                        ih * 1024 + iq * 512:
                                        ih * 1024 + (iq + 1) * 512],
                        start=True, stop=True)
                pr = probs_pool.tile([P, 1024], bf16, tag="probs", name="pr")
                nc.scalar.activation(out=pr, in_=sc, func=Act.Exp, scale=0.125)
                probs_tiles[(ih, pair, jt, hp)] = pr
                vclock[0] += mm_cost(1024, 2)
                if DEBUG and (ih, pair, jt, hp) == (0, 0, 0, 0):
                    nc.sync.dma_start(out=dbg['pr000'][:, :], in_=pr)

        # PV + denominators run as drain units after a pair's scores finish.
        # PSUM accumulation rule: only one in-flight chain per bank — so each
        # (hp, ic) region's jt-chain runs to completion before the next
        # starts (pv chain and denom chain interleave but live in different
        # banks).
        dstate = {}

        def drain_unit(ih, pair, hp, ic):
            key = (ih, pair)
            if key not in dstate:
                acc = [psum.tile([P, 8, DK], f32, tag="pvacc", bufs=2,
                                 name=f"pva{hp2}") for hp2 in range(2)]
                rec = rec_pool.tile([P, 16], f32, tag="rec", name="rec")
                dstate[key] = (acc, rec)
            acc, rec = dstate[key]
            h = 2 * pair + hp
            den = psum.tile([P, 1], f32, tag="work", bufs=2, name="den")
            for jt in range(ST):
                vproj(jt)
                pr = probs_tiles[(ih, pair, jt, hp)]
                nc.tensor.matmul(
                    out=acc[hp][:, ic],
                    lhsT=pr[:, ic * P:(ic + 1) * P],
                    rhs=v_sb[:, jt, h],
                    start=(jt == 0), stop=(jt == ST - 1))
                nc.tensor.matmul(
                    out=den,
                    lhsT=pr[:, ic * P:(ic + 1) * P],
                    rhs=ones_sb,
                    start=(jt == 0), stop=(jt == ST - 1))
            nc.vector.reciprocal(
                out=rec[:, hp * 8 + ic:hp * 8 + ic + 1], in_=den)
            vclock[0] += mm_cost(ST * DK + ST, 2 * ST)

        def boundary(ih, pair):
            acc, rec = dstate.pop((ih, pair))
            if DEBUG and (ih, pair) == (0, 0):
                nc.sync.dma_start(out=dbg['rec00'][:, :], in_=rec)
                pvd = singles.tile([P, 2, 8, DK], f32, name="pvd")
                for hp in range(2):
                    nc.vector.tensor_copy(out=pvd[:, hp], in_=acc[hp])
                nc.sync.dma_start(
                    out=dbg['pv00'][:, :],
                    in_=pvd.rearrange("p a b c -> p (a b c)"))
            atiles = []
            for ic in range(8):
                at = attn_pool.tile([P, P], bf16, tag="attn", name="at")
                for hp in range(2):
                    nc.vector.tensor_scalar_mul(
                        out=at[:, hp * DK:(hp + 1) * DK],
                        in0=acc[hp][:, ic],
                        scalar1=rec[:, hp * 8 + ic:hp * 8 + ic + 1])
                atiles.append(at)
            for ic in range(8):
                tp = psum.tile([P, P], bf16, tag="work", bufs=2, name="tp")
                nc.tensor.transpose(out=tp, in_=atiles[ic], identity=ident_sb)
                col = (ih * 8 + ic) * P
                nc.vector.tensor_copy(out=attT[pair][:, col:col + P], in_=tp)
            for jt in range(ST):
                for hp in range(2):
                    del probs_tiles[(ih, pair, jt, hp)]
            vclock[0] += mm_cost(8 * P, 8)

        def outproj(st):
            osb = outs_pool.tile([P, D], bf16, tag="osb", name="osb")
            for nb in range(2):
                po = psum.tile([P, 512], f32, tag="work", bufs=2, name="po")
                for c in range(2):
                    nc.tensor.matmul(
                        out=po,
                        lhsT=attT[c][:, st * P:(st + 1) * P],
                        rhs=wo_sb[:, c, nb * 512:(nb + 1) * 512],
                        start=(c == 0), stop=(c == 1))
                nc.vector.tensor_copy(out=osb[:, nb * 512:(nb + 1) * 512],
                                      in_=po)
            nc.sync.dma_start(out=out[st * P:(st + 1) * P, :], in_=osb)
            vclock[0] += mm_cost(2 * 512, 4)

        # ---------------- schedule ----------------
        STEP_NS = 2080.0

        # fillers: (deadline_step, emit_fn), roughly deadline-sorted
        fillers = deque()
        for g in range(1, 8):
            fillers.append((2 * g - 1, lambda g=g: qkproj('k', 0, g)))
        for sl in range(4):
            fillers.append((12, lambda sl=sl: qkproj('q', 1, sl)))
        # xk ring (5 slots): slices 5..7 reuse slots of 0..2, so kproj t1
        # for early slices must clear before the late DMAs arrive
        for g in range(4):
            fillers.append((2 + 2 * g, lambda g=g: qkproj('k', 1, g)))
        for g in range(4, 8):
            fillers.append((15 + 2 * (g - 4), lambda g=g: qkproj('k', 1, g)))
        # xv ring (4 slots): slice v+4's DMA needs vproj(2v),(2v+1) done
        for jt in range(8):
            fillers.append((4 + (jt * 3) // 4, lambda jt=jt: vproj(jt)))
        for jt in range(8, ST):
            fillers.append((15, lambda jt=jt: vproj(jt)))
        for sl in range(4, 8):
            fillers.append((29, lambda sl=sl: qkproj('q', 0, sl)))
            fillers.append((30, lambda sl=sl: qkproj('q', 1, sl)))
        fillers = deque(sorted(fillers, key=lambda x: x[0]))
        late_fillers = deque()   # outproj st0-7, enabled after ih0 done

        # prologue
        for sl in range(4):
            qkproj('q', 0, sl)
        qkproj('k', 0, 0)

        steps = [(ih, pair, jt) for ih in range(2) for pair in range(2)
                 for jt in range(ST)]
        drainq = deque()        # (enq_step, ih, pair, hp, ic)
        remaining = {}          # (ih,pair) -> remaining drain units

        def pump(step_idx):
            # forced: overdue fillers
            while fillers and fillers[0][0] <= step_idx:
                fillers.popleft()[1]()
            # greedy: stay ahead of the ACT pace
            while (fillers or late_fillers) and \
                    vclock[0] < (step_idx + 1) * STEP_NS * 0.92:
                if fillers:
                    fillers.popleft()[1]()
                else:
                    late_fillers.popleft()()

        def pop_drains(si, limit=3):
            pops = 0
            while drainq and drainq[0][0] < si and pops < limit:
                _, dih, dpair, hp, ic = drainq.popleft()
                drain_unit(dih, dpair, hp, ic)
                pops += 1
                remaining[(dih, dpair)] -= 1
                if remaining[(dih, dpair)] == 0:
                    boundary(dih, dpair)
                    if (dih, dpair) == (0, 1):
                        for st in range(8):
                            late_fillers.append(lambda st=st: outproj(st))

        for si, (ih, pair, jt) in enumerate(steps):
            scores_exp(ih, pair, jt)
            if jt == ST - 1:
                remaining[(ih, pair)] = 16
                for hp in range(2):
                    for ic in range(8):
                        drainq.append((si, ih, pair, hp, ic))
            pop_drains(si)
            pump(si)

        # tail: drain the last pair, then the remaining output projections
        si = len(steps)
        while drainq:
            pop_drains(si + 1000, limit=100)
        while late_fillers:
            late_fillers.popleft()()
        for st in range(8, 16):
            outproj(st)
        if DEBUG:
            nc.sync.dma_start(out=dbg['qT0'][:, :], in_=qT_sb[0])
            nc.sync.dma_start(out=dbg['kT0'][:, :], in_=kT_sb[0])
            nc.sync.dma_start(
                out=dbg['v'][:, :],
                in_=v_sb.rearrange("p a b c -> p (a b c)"))
            nc.sync.dma_start(out=dbg['attT0'][:, :], in_=attT[0])
            nc.sync.dma_start(out=dbg['attT1'][:, :], in_=attT[1])

    nc.finalize()
    return nc


def _pack_x(Xb):
    # [S, D] f32 -> [128, NSL*KC*SLW] bf16, slices sl-major, (c, s) inside
    a = Xb.reshape(NSL, SLW, KC, P).transpose(3, 0, 2, 1)
    return np.ascontiguousarray(a).reshape(P, NSL * KC * SLW).astype(BF16)


def kernel(Q, K, V, Wq, bq, Wk, bk, Wv, bv, Wo, bo):
    from concourse.bass_utils import run_bass_kernel_spmd

    f32 = np.float32
    Q = np.asarray(Q, f32)
    K = np.asarray(K, f32)
    V = np.asarray(V, f32)
    Wq = np.asarray(Wq, f32)
    Wk = np.asarray(Wk, f32)
    Wv = np.asarray(Wv, f32)
    Wo = np.asarray(Wo, f32)
    bq = np.asarray(bq, f32)
    bk = np.asarray(bk, f32)
    bv = np.asarray(bv, f32)
    bo = np.asarray(bo, f32)

    xp = {}
    for b in range(B):
        xp[('q', b)] = _pack_x(Q[b])
        xp[('k', b)] = _pack_x(K[b])
        xp[('v', b)] = _pack_x(V[b])

    ident = np.eye(P, dtype=BF16)

    def pack_w(Wslice):
        # [1024, 256] -> [128, KC, 256] (p, c, m) -> flat
        a = Wslice.reshape(KC, P, GD).transpose(1, 0, 2)
        return np.ascontiguousarray(a).reshape(P, KC * GD).astype(BF16)

    in_maps = []
    for c in range(NCORES):
        b, g = c // GROUPS, c % GROUPS
        sl = slice(g * GD, (g + 1) * GD)
        wo_a = Wo[sl, :].reshape(2, P, D).transpose(1, 0, 2)
        in_maps.append({
            "xq": xp[('q', b)],
            "xk": xp[('k', b)],
            "xv": xp[('v', b)],
            "wq": pack_w(Wq[:, sl]),
            "wk": pack_w(Wk[:, sl]),
            "wv": pack_w(Wv[:, sl]),
            "wo": np.ascontiguousarray(wo_a).reshape(P, 2 * D).astype(BF16),
            "bq": np.ascontiguousarray(bq[sl].reshape(2, P).T),
            "bk": np.ascontiguousarray(bk[sl].reshape(2, P).T),
            "ident": ident,
        })

    if "nc" not in _cached:
        _cached["nc"] = _build_bass()
    nc = _cached["nc"]

    try:
        res = run_bass_kernel_spmd(nc, in_maps, core_ids=list(range(NCORES)))
    except ModuleNotFoundError:
        os.environ["BASS_NEVER_TRACE"] = "1"
        res = run_bass_kernel_spmd(nc, in_maps, core_ids=list(range(NCORES)))
    if res.exec_time_ns is not None:
        print(f"HW exec time: {res.exec_time_ns} ns")

    bo_eff = (bv @ Wo + bo).astype(f32)
    out = np.zeros((B, S, D), f32)
    for c in range(NCORES):
        b = c // GROUPS
        out[b] += res.results[c]["out"].astype(f32)
    out += bo_eff
    return out


# revision 51
# speedup vs baseline: 1.3480x; 1.0767x over previous
"""Multi-head attention (B=2, S=2048, D=1024, H=16, dk=64) on 8 TRN2 cores.

Sharding: core c -> (batch b = c//4, head-group g = c%4 of 4 heads).

Design (v2, ACT-paced pipeline):
  The softmax exp is the hard floor: 4 heads x 2048^2 scores / 128 lanes
  on the one engine that can exp (ACT) ~= 133us.  Everything else is
  scheduled to hide under it:
  - inputs stream in 256-column s-slices (full-bandwidth 4KB runs) so the
    first exp fires at ~11us instead of ~26us;
  - scoresT[j,i] -> exp -> probs[j,i] (bf16, SBUF);
  - PV in [i,e] orientation: out[i,64] = probs[j,i-chunk]^T @ v[j,64],
    F=64 accumulating matmuls (half the PE rows of the [e,i] form);
  - softmax denominators recomputed at each head-pair boundary from the
    buffered probs tiles (F=1 accumulating matmuls into a transient PSUM
    tile), reciprocal + per-partition scalar-mul normalize on DVE;
  - normalized attn [i,(hp,e)] transposed per 128-chunk on the PE
    (one [128,128] transpose covers both heads), giving attT[e,s] for a
    K=128x2 output projection;
  - out projection for the first s-half runs inside the second half's
    attention; bf16 partial outputs (host reduces in f32).
  PSUM: scores 2x[128,1024](4 banks) + pvacc 2x[128,512](2) + work ring
  2x[128,512](2) = 8 banks exactly.
"""

import os
from collections import deque

import numpy as np
import ml_dtypes

BF16 = ml_dtypes.bfloat16

B, S, D = 2, 2048, 1024
H, DK = 16, 64
P = 128
GROUPS = 4
HPG = 4             # heads per group (2 pairs)
GD = HPG * DK       # 256
KC = D // P         # 8 contraction chunks
NSL = 8             # 256-wide s-slices per tensor
SLW = S // NSL      # 256
ST = S // P         # 16 j-tiles / s-tiles
NCORES = 8
PV_LAG = 5          # steps between exp(jt) and pv(jt)

_cached = {}
DEBUG = False


def _build_bass():
    import concourse.bass as bass
    import concourse.tile as tile
    from concourse.bacc import Bacc
    from concourse import mybir
    from contextlib import ExitStack

    f32 = mybir.dt.float32
    bf16 = mybir.dt.bfloat16
    Act = mybir.ActivationFunctionType

    nc = Bacc()

    xq = nc.dram_tensor("xq", [P, NSL * KC * SLW], bf16, kind="ExternalInput")
    xk = nc.dram_tensor("xk", [P, NSL * KC * SLW], bf16, kind="ExternalInput")
    xv = nc.dram_tensor("xv", [P, NSL * KC * SLW], bf16, kind="ExternalInput")
    wq = nc.dram_tensor("wq", [P, KC * GD], bf16, kind="ExternalInput")
    wk = nc.dram_tensor("wk", [P, KC * GD], bf16, kind="ExternalInput")
    wv = nc.dram_tensor("wv", [P, KC * GD], bf16, kind="ExternalInput")
    wo = nc.dram_tensor("wo", [P, 2 * D], bf16, kind="ExternalInput")
    bq = nc.dram_tensor("bq", [P, 2], f32, kind="ExternalInput")
    bk = nc.dram_tensor("bk", [P, 2], f32, kind="ExternalInput")
    ident = nc.dram_tensor("ident", [P, P], bf16, kind="ExternalInput")
    out = nc.dram_tensor("out", [S, D], bf16, kind="ExternalOutput")
    if DEBUG:
        dbg = {
            'qT0': nc.dram_tensor("d_qT0", [P, S], bf16, kind="ExternalOutput"),
            'kT0': nc.dram_tensor("d_kT0", [P, S], bf16, kind="ExternalOutput"),
            'v': nc.dram_tensor("d_v", [P, ST * HPG * DK], bf16,
                                kind="ExternalOutput"),
            'pr000': nc.dram_tensor("d_pr000", [P, 1024], bf16,
                                    kind="ExternalOutput"),
            'rec00': nc.dram_tensor("d_rec00", [P, 16], f32,
                                    kind="ExternalOutput"),
            'pv00': nc.dram_tensor("d_pv00", [P, 2 * 8 * DK], f32,
                                   kind="ExternalOutput"),
            'attT0': nc.dram_tensor("d_attT0", [P, S], bf16,
                                    kind="ExternalOutput"),
            'attT1': nc.dram_tensor("d_attT1", [P, S], bf16,
                                    kind="ExternalOutput"),
        }

    with tile.TileContext(nc) as tc, ExitStack() as ctx:
        singles = ctx.enter_context(tc.tile_pool(name="singles", bufs=1))
        xring = ctx.enter_context(tc.tile_pool(name="xring", bufs=4))
        probs_pool = ctx.enter_context(tc.tile_pool(name="probs", bufs=43))
        attn_pool = ctx.enter_context(tc.tile_pool(name="attn", bufs=8))
        rec_pool = ctx.enter_context(tc.tile_pool(name="rec", bufs=2))
        outs_pool = ctx.enter_context(tc.tile_pool(name="outs", bufs=4))
        psum = ctx.enter_context(tc.tile_pool(name="psum", bufs=1, space="PSUM"))

        # ---------------- persistent SBUF ----------------
        wq_sb = singles.tile([P, KC, GD], bf16)
        wk_sb = singles.tile([P, KC, GD], bf16)
        wv_sb = singles.tile([P, KC, GD], bf16)
        wo_sb = singles.tile([P, 2, D], bf16)
        bq_sb = singles.tile([P, 2], f32)
        bk_sb = singles.tile([P, 2], f32)
        ident_sb = singles.tile([P, P], bf16)
        ones_sb = singles.tile([P, 1], bf16)
        dummy_sb = singles.tile([P, 1], f32)

        qT_sb = [singles.tile([P, S], bf16, name=f"qT{t}") for t in range(2)]
        kT_sb = [singles.tile([P, S], bf16, name=f"kT{t}") for t in range(2)]
        v_sb = singles.tile([P, ST, HPG, DK], bf16)
        attT = [singles.tile([P, S], bf16, name=f"attT{p}") for p in range(2)]

        # warm the ACT exp table during the DMA lead-in
        nc.vector.memset(ones_sb, 1.0)
        nc.vector.memset(dummy_sb, 0.0)
        warm = singles.tile([P, 1], f32)
        nc.scalar.activation(out=warm, in_=dummy_sb, func=Act.Exp)

        # ---------------- DMA stream (order = schedule) ----------------
        nc.sync.dma_start(out=wq_sb, in_=wq.rearrange("p (c m) -> p c m", c=KC))
        nc.sync.dma_start(out=wk_sb, in_=wk.rearrange("p (c m) -> p c m", c=KC))

        # x slice rings: slot = [P, KC, SLW] bf16 (4KB/partition)
        x_slots = {}

        def dma_x(which, dram, sl):
            t = xring.tile([P, KC, SLW], bf16, tag=f"x{which}",
                           bufs={'q': 3, 'k': 8, 'v': 4}[which],
                           name=f"x{which}{sl}")
            nc.sync.dma_start(
                out=t, in_=dram.rearrange("p (sl c s) -> p sl c s",
                                          sl=NSL, c=KC)[:, sl])
            x_slots[(which, sl)] = t

        for sl in range(4):
            dma_x('q', xq, sl)
        dma_x('k', xk, 0)
        nc.sync.dma_start(out=bq_sb, in_=bq[:, :])
        nc.sync.dma_start(out=bk_sb, in_=bk[:, :])
        nc.sync.dma_start(out=wv_sb, in_=wv.rearrange("p (c m) -> p c m", c=KC))
        for g in range(1, 8):
            dma_x('k', xk, g)
        nc.sync.dma_start(out=ident_sb, in_=ident[:, :])
        for g in range(8):
            dma_x('v', xv, g)
        for sl in range(4, 8):
            dma_x('q', xq, sl)
        nc.sync.dma_start(out=wo_sb, in_=wo.rearrange("p (c n) -> p c n", c=2))

        # ---------------- unit emitters (memoized) ----------------
        emitted = set()
        vclock = [0.0]     # virtual PE ns

        def mm_cost(rows, n=1):
            return rows * 0.42 + n * 5.0

        def qkproj(which, t, sl):
            key = (which, t, sl)
            if key in emitted:
                return
            emitted.add(key)
            x_t = x_slots[(which, sl)]
            w_sb = wq_sb if which == 'q' else wk_sb
            b_sb = bq_sb if which == 'q' else bk_sb
            dstT = (qT_sb if which == 'q' else kT_sb)[t]
            pp = psum.tile([P, 512], f32, tag="work", bufs=2, name="pp")
            for c in range(KC):
                nc.tensor.matmul(out=pp[:, 0:SLW],
                                 lhsT=w_sb[:, c, t * P:(t + 1) * P],
                                 rhs=x_t[:, c, :],
                                 start=(c == 0), stop=(c == KC - 1))
            nc.vector.tensor_scalar_add(
                out=dstT[:, sl * SLW:(sl + 1) * SLW],
                in0=pp[:, 0:SLW], scalar1=b_sb[:, t:t + 1])
            vclock[0] += mm_cost(KC * SLW, KC)

        def vproj(jt, pair=None):
            key = ('v', jt)
            if key in emitted:
                return
            emitted.add(key)
            sl, half = jt // 2, jt % 2
            x_t = x_slots[('v', sl)]
            pp = psum.tile([P, 512], f32, tag="work", bufs=2, name="pv")
            for c in range(KC):
                nc.tensor.matmul(
                    out=pp[:, 0:GD],
                    lhsT=x_t[:, c, half * P:(half + 1) * P],
                    rhs=wv_sb[:, c, :],
                    start=(c == 0), stop=(c == KC - 1))
            nc.vector.tensor_copy(
                out=v_sb[:, jt], in_=pp[:, 0:GD].rearrange("p (h e) -> p h e",
                                                           e=DK))
            vclock[0] += mm_cost(KC * GD, KC)

        probs_tiles = {}
        pvaccs = {}
        sc_seq = [0]

        def scores_exp(ih, pair, jt):
            qkproj('k', pair, jt // 2)
            for sl in range(ih * 4, ih * 4 + 4):
                qkproj('q', pair, sl)
            for hp in range(2):
                sc = psum.tile([P, 1024], f32, tag="sc", bufs=2,
                               name=f"sc{sc_seq[0] % 2}")
                sc_seq[0] += 1
                for iq in range(2):
                    nc.tensor.matmul(
                        out=sc[:, iq * 512:(iq + 1) * 512],
                        lhsT=kT_sb[pair][hp * DK:(hp + 1) * DK,
                                         jt * P:(jt + 1) * P],
                        rhs=qT_sb[pair][hp * DK:(hp + 1) * DK,
                                        ih * 1024 + iq * 512:
                                        ih * 1024 + (iq + 1) * 512],
                        start=True, stop=True)
                pr = probs_pool.tile([P, 1024], bf16, tag="probs", name="pr")
                nc.scalar.activation(out=pr, in_=sc, func=Act.Exp, scale=0.125)
                probs_tiles[(ih, pair, jt, hp)] = pr
                vclock[0] += mm_cost(1024, 2)
                if DEBUG and (ih, pair, jt, hp) == (0, 0, 0, 0):
                    nc.sync.dma_start(out=dbg['pr000'][:, :], in_=pr)

        # PV + denominators run as drain units after a pair's scores finish.
        # PSUM accumulation rule: only one in-flight chain per bank — so each
        # (hp, ic) region's jt-chain runs to completion before the next
        # starts (pv chain and denom chain interleave but live in different
        # banks).
        dstate = {}
        in_tail = [False]   # when True (ACT idle), split copies ACT/DVE

        def drain_unit(ih, pair, hp, ic):
            key = (ih, pair)
            if key not in dstate:
                acc = [psum.tile([P, 8, DK], f32, tag="pvacc", bufs=2,
                                 name=f"pva{hp2}") for hp2 in range(2)]
                rec = rec_pool.tile([P, 16], f32, tag="rec", name="rec")
                dstate[key] = (acc, rec)
            acc, rec = dstate[key]
            h = 2 * pair + hp
            den = psum.tile([P, 1], f32, tag="work", bufs=2, name="den")
            for jt in range(ST):
                vproj(jt, pair)
                pr = probs_tiles[(ih, pair, jt, hp)]
                nc.tensor.matmul(
                    out=acc[hp][:, ic],
                    lhsT=pr[:, ic * P:(ic + 1) * P],
                    rhs=v_sb[:, jt, h],
                    start=(jt == 0), stop=(jt == ST - 1))
                nc.tensor.matmul(
                    out=den,
                    lhsT=pr[:, ic * P:(ic + 1) * P],
                    rhs=ones_sb,
                    start=(jt == 0), stop=(jt == ST - 1))
            nc.vector.reciprocal(
                out=rec[:, hp * 8 + ic:hp * 8 + ic + 1], in_=den)
            vclock[0] += mm_cost(ST * DK + ST, 2 * ST)

        def finish_ic(ih, pair, ic):
            # normalize + transpose one 128-column block (both heads of the
            # pair); in the tail also fire its output projection immediately
            acc, rec = dstate[(ih, pair)]
            at = attn_pool.tile([P, P], bf16, tag="attn", name="at")
            for hp in range(2):
                r = rec[:, hp * 8 + ic:hp * 8 + ic + 1]
                if in_tail[0] and hp == 1:
                    nc.scalar.mul(out=at[:, hp * DK:(hp + 1) * DK],
                                  in_=acc[hp][:, ic], mul=r)
                else:
                    nc.vector.tensor_scalar_mul(
                        out=at[:, hp * DK:(hp + 1) * DK],
                        in0=acc[hp][:, ic], scalar1=r)
            tp = psum.tile([P, P], bf16, tag="work", bufs=2, name="tp")
            nc.tensor.transpose(out=tp, in_=at, identity=ident_sb)
            col = (ih * 8 + ic) * P
            if in_tail[0] and ic % 2 == 1:
                nc.scalar.copy(out=attT[pair][:, col:col + P], in_=tp)
            else:
                nc.vector.tensor_copy(out=attT[pair][:, col:col + P], in_=tp)
            if in_tail[0]:
                outproj(ih * 8 + ic)
            vclock[0] += mm_cost(P, 1)

        def boundary(ih, pair):
            acc, rec = dstate.pop((ih, pair))
            if DEBUG and (ih, pair) == (0, 0):
                nc.sync.dma_start(out=dbg['rec00'][:, :], in_=rec)
                pvd = singles.tile([P, 2, 8, DK], f32, name="pvd")
                for hp in range(2):
                    nc.vector.tensor_copy(out=pvd[:, hp], in_=acc[hp])
                nc.sync.dma_start(
                    out=dbg['pv00'][:, :],
                    in_=pvd.rearrange("p a b c -> p (a b c)"))
            for jt in range(ST):
                for hp in range(2):
                    del probs_tiles[(ih, pair, jt, hp)]

        def outproj(st):
            osb = outs_pool.tile([P, D], bf16, tag="osb", name="osb")
            for nb in range(2):
                # in the tail the sc ring is idle; using it keeps the work
                # ring free for the drain/transpose tiles
                po = psum.tile([P, 512], f32,
                               tag="sc" if in_tail[0] else "work",
                               bufs=2, name="po")
                for c in range(2):
                    nc.tensor.matmul(
                        out=po,
                        lhsT=attT[c][:, st * P:(st + 1) * P],
                        rhs=wo_sb[:, c, nb * 512:(nb + 1) * 512],
                        start=(c == 0), stop=(c == 1))
                if in_tail[0] and nb == 0:
                    nc.scalar.copy(out=osb[:, nb * 512:(nb + 1) * 512],
                                   in_=po)
                else:
                    nc.vector.tensor_copy(out=osb[:, nb * 512:(nb + 1) * 512],
                                          in_=po)
            nc.sync.dma_start(out=out[st * P:(st + 1) * P, :], in_=osb)
            vclock[0] += mm_cost(2 * 512, 4)

        # ---------------- schedule ----------------
        STEP_NS = 2080.0

        # fillers: (deadline_step, emit_fn), roughly deadline-sorted
        fillers = deque()
        for g in range(1, 8):
            fillers.append((2 * g - 1, lambda g=g: qkproj('k', 0, g)))
        for sl in range(4):
            fillers.append((12, lambda sl=sl: qkproj('q', 1, sl)))
        fillers.append((13, lambda: qkproj('k', 1, 0)))
        for g in range(1, 8):
            fillers.append((14 + 2 * g, lambda g=g: qkproj('k', 1, g)))
        # xv ring (4 slots): slice v+4 arrives ~step 8 and reuses slot 0
        for jt in range(ST):
            fillers.append((8 + jt // 2, lambda jt=jt: vproj(jt)))
        # xq ring (3 slots): slices must be consumed in slice order across
        # BOTH t's (ring recycling), so keep each slice's two units adjacent
        for sl in range(4, 8):
            fillers.append((22 + sl, lambda sl=sl: qkproj('q', 0, sl)))
            fillers.append((22 + sl, lambda sl=sl: qkproj('q', 1, sl)))
        fillers = deque(sorted(fillers, key=lambda x: x[0]))
        late_fillers = deque()   # outproj st0-7, enabled after ih0 done

        # prologue (xq ring has 3 slots: slice 3's DMA needs slice-0
        # consumers done, so project both t's for slice 0 first)
        qkproj('q', 0, 0)
        qkproj('q', 1, 0)
        for sl in range(1, 4):
            qkproj('q', 0, sl)
        qkproj('k', 0, 0)

        steps = [(ih, pair, jt) for ih in range(2) for pair in range(2)
                 for jt in range(ST)]
        drainq = deque()        # (enq_step, ih, pair, hp, ic)
        remaining = {}          # (ih,pair) -> remaining drain units

        def pump(step_idx):
            # forced: overdue fillers
            while fillers and fillers[0][0] <= step_idx:
                fillers.popleft()[1]()
            # greedy: stay only slightly ahead of the ACT pace so movable
            # work lands in the late-attention PE slack instead of the
            # early crunch
            while (fillers or late_fillers) and \
                    vclock[0] < (step_idx + 1) * STEP_NS * 0.80:
                if fillers:
                    fillers.popleft()[1]()
                else:
                    late_fillers.popleft()()

        pending_fin = deque()   # software-pipeline finish_ic one ic behind

        def pop_drains(si, limit=3):
            pops = 0
            while drainq and drainq[0][0] < si and pops < limit:
                _, dih, dpair, hp, ic = drainq.popleft()
                drain_unit(dih, dpair, hp, ic)
                pops += 1
                if hp == 1:
                    pending_fin.append((dih, dpair, ic))
                    if len(pending_fin) > 1:
                        finish_ic(*pending_fin.popleft())
                remaining[(dih, dpair)] -= 1
                if remaining[(dih, dpair)] == 0:
                    while pending_fin:
                        finish_ic(*pending_fin.popleft())
                    boundary(dih, dpair)
                    if (dih, dpair) == (0, 1):
                        for st in range(8):
                            late_fillers.append(lambda st=st: outproj(st))

        for si, (ih, pair, jt) in enumerate(steps):
            scores_exp(ih, pair, jt)
            if jt == ST - 1:
                remaining[(ih, pair)] = 16
                for ic in range(8):
                    for hp in range(2):
                        drainq.append((si, ih, pair, hp, ic))
            pop_drains(si)
            if si >= 40 and late_fillers:
                late_fillers.popleft()()
            pump(si)

        # tail: drain the last pair; its boundary emits outproj per-ic
        in_tail[0] = True
        si = len(steps)
        while late_fillers:
            late_fillers.popleft()()
        while drainq:
            pop_drains(si + 1000, limit=100)
        if DEBUG:
            nc.sync.dma_start(out=dbg['qT0'][:, :], in_=qT_sb[0])
            nc.sync.dma_start(out=dbg['kT0'][:, :], in_=kT_sb[0])
            nc.sync.dma_start(
                out=dbg['v'][:, :],
                in_=v_sb.rearrange("p a b c -> p (a b c)"))
            nc.sync.dma_start(out=dbg['attT0'][:, :], in_=attT[0])
            nc.sync.dma_start(out=dbg['attT1'][:, :], in_=attT[1])

    nc.finalize()
    return nc


def _pack_x(Xb):
    # [S, D] f32 -> [128, NSL*KC*SLW] bf16, slices sl-major, (c, s) inside
    a = Xb.reshape(NSL, SLW, KC, P).transpose(3, 0, 2, 1)
    return np.ascontiguousarray(a).reshape(P, NSL * KC * SLW).astype(BF16)


def kernel(Q, K, V, Wq, bq, Wk, bk, Wv, bv, Wo, bo):
    from concourse.bass_utils import run_bass_kernel_spmd

    f32 = np.float32
    Q = np.asarray(Q, f32)
    K = np.asarray(K, f32)
    V = np.asarray(V, f32)
    Wq = np.asarray(Wq, f32)
    Wk = np.asarray(Wk, f32)
    Wv = np.asarray(Wv, f32)
    Wo = np.asarray(Wo, f32)
    bq = np.asarray(bq, f32)
    bk = np.asarray(bk, f32)
    bv = np.asarray(bv, f32)
    bo = np.asarray(bo, f32)

    xp = {}
    for b in range(B):
        xp[('q', b)] = _pack_x(Q[b])
        xp[('k', b)] = _pack_x(K[b])
        xp[('v', b)] = _pack_x(V[b])

    ident = np.eye(P, dtype=BF16)

    def pack_w(Wslice):
        # [1024, 256] -> [128, KC, 256] (p, c, m) -> flat
        a = Wslice.reshape(KC, P, GD).transpose(1, 0, 2)
        return np.ascontiguousarray(a).reshape(P, KC * GD).astype(BF16)

    in_maps = []
    for c in range(NCORES):
        b, g = c // GROUPS, c % GROUPS
        sl = slice(g * GD, (g + 1) * GD)
        wo_a = Wo[sl, :].reshape(2, P, D).transpose(1, 0, 2)
        in_maps.append({
            "xq": xp[('q', b)],
            "xk": xp[('k', b)],
            "xv": xp[('v', b)],
            "wq": pack_w(Wq[:, sl]),
            "wk": pack_w(Wk[:, sl]),
            "wv": pack_w(Wv[:, sl]),
            "wo": np.ascontiguousarray(wo_a).reshape(P, 2 * D).astype(BF16),
            "bq": np.ascontiguousarray(bq[sl].reshape(2, P).T),
            "bk": np.ascontiguousarray(bk[sl].reshape(2, P).T),
            "ident": ident,
        })

    if "nc" not in _cached:
        _cached["nc"] = _build_bass()
    nc = _cached["nc"]

    try:
        res = run_bass_kernel_spmd(nc, in_maps, core_ids=list(range(NCORES)))
    except ModuleNotFoundError:
        os.environ["BASS_NEVER_TRACE"] = "1"
        res = run_bass_kernel_spmd(nc, in_maps, core_ids=list(range(NCORES)))
    if res.exec_time_ns is not None:
        print(f"HW exec time: {res.exec_time_ns} ns")

    bo_eff = (bv @ Wo + bo).astype(f32)
    out = np.zeros((B, S, D), f32)
    for c in range(NCORES):
        b = c // GROUPS
        out[b] += res.results[c]["out"].astype(f32)
    out += bo_eff
    return out


# revision 95
# speedup vs baseline: 1.3964x; 1.0359x over previous
"""Multi-head attention (B=2, S=2048, D=1024, H=16, dk=64) on 8 TRN2 cores.

Sharding: core c -> (batch b = c//4, head-group g = c%4 of 4 heads).

Design (v2, ACT-paced pipeline):
  The softmax exp is the hard floor: 4 heads x 2048^2 scores / 128 lanes
  on the one engine that can exp (ACT) ~= 133us.  Everything else is
  scheduled to hide under it:
  - inputs stream in 256-column s-slices (full-bandwidth 4KB runs) so the
    first exp fires at ~11us instead of ~26us;
  - scoresT[j,i] -> exp -> probs[j,i] (bf16, SBUF);
  - PV in [i,e] orientation: out[i,64] = probs[j,i-chunk]^T @ v[j,64],
    F=64 accumulating matmuls (half the PE rows of the [e,i] form);
  - softmax denominators recomputed at each head-pair boundary from the
    buffered probs tiles (F=1 accumulating matmuls into a transient PSUM
    tile), reciprocal + per-partition scalar-mul normalize on DVE;
  - normalized attn [i,(hp,e)] transposed per 128-chunk on the PE
    (one [128,128] transpose covers both heads), giving attT[e,s] for a
    K=128x2 output projection;
  - out projection for the first s-half runs inside the second half's
    attention; bf16 partial outputs (host reduces in f32).
  PSUM: scores 2x[128,1024](4 banks) + pvacc 2x[128,512](2) + work ring
  2x[128,512](2) = 8 banks exactly.
"""

import os
from collections import deque

import numpy as np
import ml_dtypes

BF16 = ml_dtypes.bfloat16

B, S, D = 2, 2048, 1024
H, DK = 16, 64
P = 128
GROUPS = 4
HPG = 4             # heads per group (2 pairs)
GD = HPG * DK       # 256
KC = D // P         # 8 contraction chunks
NSL = 8             # 256-wide s-slices per tensor
SLW = S // NSL      # 256
ST = S // P         # 16 j-tiles / s-tiles
NCORES = 8
PV_LAG = 5          # steps between exp(jt) and pv(jt)

_cached = {}
DEBUG = False
# schedule knobs (emission order only — numerics-neutral)
KNOBS = {
    'step_budget': 1800.0,  # ns of non-score PE work per step
    'forced_age': 1,        # steps overdue before a filler bypasses budget
    'drain_age': 3,         # drain age that bypasses budget (probs ring)
    'drain_cap': 6,         # max drain units per step
    'q47_t0': 8,            # deadline base for qproj t0 slices 4-7 (+sl)
    'q47_t1': 38,           # deadline base for qproj t1 slices 4-7 (+sl)
    'late_start': 52,       # step to start outproj st0-7
    'vp_base': 9,           # vproj jt<12 deadline base (+jt//2)
}


def _build_bass():
    import concourse.bass as bass
    import concourse.tile as tile
    from concourse.bacc import Bacc
    from concourse import mybir
    from contextlib import ExitStack

    f32 = mybir.dt.float32
    bf16 = mybir.dt.bfloat16
    Act = mybir.ActivationFunctionType

    nc = Bacc()

    xq = nc.dram_tensor("xq", [P, NSL * KC * SLW], bf16, kind="ExternalInput")
    xk = nc.dram_tensor("xk", [P, NSL * KC * SLW], bf16, kind="ExternalInput")
    xv = nc.dram_tensor("xv", [P, NSL * KC * SLW], bf16, kind="ExternalInput")
    wq = nc.dram_tensor("wq", [P, KC * GD], bf16, kind="ExternalInput")
    wk = nc.dram_tensor("wk", [P, KC * GD], bf16, kind="ExternalInput")
    wv = nc.dram_tensor("wv", [P, KC * GD], bf16, kind="ExternalInput")
    wo = nc.dram_tensor("wo", [P, 2 * D], bf16, kind="ExternalInput")
    bq = nc.dram_tensor("bq", [P, 2], f32, kind="ExternalInput")
    bk = nc.dram_tensor("bk", [P, 2], f32, kind="ExternalInput")
    ident = nc.dram_tensor("ident", [P, P], bf16, kind="ExternalInput")
    out = nc.dram_tensor("out", [S, D], bf16, kind="ExternalOutput")
    if DEBUG:
        dbg = {
            'qT0': nc.dram_tensor("d_qT0", [P, S], bf16, kind="ExternalOutput"),
            'kT0': nc.dram_tensor("d_kT0", [P, S], bf16, kind="ExternalOutput"),
            'v': nc.dram_tensor("d_v", [P, ST * HPG * DK], bf16,
                                kind="ExternalOutput"),
            'pr000': nc.dram_tensor("d_pr000", [P, 1024], bf16,
                                    kind="ExternalOutput"),
            'rec00': nc.dram_tensor("d_rec00", [P, 16], f32,
                                    kind="ExternalOutput"),
            'pv00': nc.dram_tensor("d_pv00", [P, 2 * 8 * DK], f32,
                                   kind="ExternalOutput"),
            'attT0': nc.dram_tensor("d_attT0", [P, S], bf16,
                                    kind="ExternalOutput"),
            'attT1': nc.dram_tensor("d_attT1", [P, S], bf16,
                                    kind="ExternalOutput"),
        }

    with tile.TileContext(nc) as tc, ExitStack() as ctx:
        singles = ctx.enter_context(tc.tile_pool(name="singles", bufs=1))
        xring = ctx.enter_context(tc.tile_pool(name="xring", bufs=4))
        probs_pool = ctx.enter_context(tc.tile_pool(name="probs", bufs=40))
        attn_pool = ctx.enter_context(tc.tile_pool(name="attn", bufs=6))
        rec_pool = ctx.enter_context(tc.tile_pool(name="rec", bufs=2))
        outs_pool = ctx.enter_context(tc.tile_pool(name="outs", bufs=3))
        psum = ctx.enter_context(tc.tile_pool(name="psum", bufs=1, space="PSUM"))

        # ---------------- persistent SBUF ----------------
        wq_sb = singles.tile([P, KC, GD], bf16)
        wk_sb = singles.tile([P, KC, GD], bf16)
        wv_sb = singles.tile([P, KC, GD], bf16)
        wo_sb = singles.tile([P, 2, D], bf16)
        bq_sb = singles.tile([P, 2], f32)
        bk_sb = singles.tile([P, 2], f32)
        ident_sb = singles.tile([P, P], bf16)
        ones_sb = singles.tile([P, 1], bf16)
        dummy_sb = singles.tile([P, 1], f32)

        qT_sb = [singles.tile([P, S], bf16, name=f"qT{t}") for t in range(2)]
        kT_sb = [singles.tile([P, S], bf16, name=f"kT{t}") for t in range(2)]
        v_sb = singles.tile([P, ST, HPG, DK], bf16)
        attT = [singles.tile([P, S], bf16, name=f"attT{p}") for p in range(2)]

        # warm the ACT exp table during the DMA lead-in
        nc.vector.memset(ones_sb, 1.0)
        nc.vector.memset(dummy_sb, 0.0)
        warm = singles.tile([P, 1], f32)
        nc.scalar.activation(out=warm, in_=dummy_sb, func=Act.Exp)

        # ---------------- DMA stream (order = schedule) ----------------
        nc.sync.dma_start(out=wq_sb, in_=wq.rearrange("p (c m) -> p c m", c=KC))

        # x slice rings: slot = [P, KC, SLW] bf16 (4KB/partition)
        x_slots = {}

        def dma_x(which, dram, sl):
            t = xring.tile([P, KC, SLW], bf16, tag=f"x{which}",
                           bufs={'q': 4, 'k': 8, 'v': 4}[which],
                           name=f"x{which}{sl}")
            nc.sync.dma_start(
                out=t, in_=dram.rearrange("p (sl c s) -> p sl c s",
                                          sl=NSL, c=KC)[:, sl])
            x_slots[(which, sl)] = t

        dma_x('q', xq, 0)
        nc.sync.dma_start(out=wk_sb, in_=wk.rearrange("p (c m) -> p c m", c=KC))
        dma_x('q', xq, 1)
        dma_x('k', xk, 0)
        nc.sync.dma_start(out=bq_sb, in_=bq[:, :])
        nc.sync.dma_start(out=bk_sb, in_=bk[:, :])
        dma_x('q', xq, 2)
        dma_x('q', xq, 3)
        dma_x('k', xk, 1)
        dma_x('k', xk, 2)
        nc.sync.dma_start(out=wv_sb, in_=wv.rearrange("p (c m) -> p c m", c=KC))
        for g in range(3, 8):
            dma_x('k', xk, g)
        nc.sync.dma_start(out=ident_sb, in_=ident[:, :])
        for g in range(8):
            dma_x('v', xv, g)
        for sl in range(4, 8):
            dma_x('q', xq, sl)
        nc.sync.dma_start(out=wo_sb, in_=wo.rearrange("p (c n) -> p c n", c=2))

        # ---------------- unit emitters (memoized) ----------------
        emitted = set()
        vclock = [0.0]     # virtual PE ns

        def mm_cost(rows, n=1):
            return rows * 0.42 + n * 5.0

        def qkproj(which, t, sl):
            key = (which, t, sl)
            if key in emitted:
                return
            emitted.add(key)
            x_t = x_slots[(which, sl)]
            w_sb = wq_sb if which == 'q' else wk_sb
            b_sb = bq_sb if which == 'q' else bk_sb
            dstT = (qT_sb if which == 'q' else kT_sb)[t]
            pp = psum.tile([P, 512], f32, tag="work", bufs=2, name="pp")
            for c in range(KC):
                nc.tensor.matmul(out=pp[:, 0:SLW],
                                 lhsT=w_sb[:, c, t * P:(t + 1) * P],
                                 rhs=x_t[:, c, :],
                                 start=(c == 0), stop=(c == KC - 1))
            nc.vector.tensor_scalar_add(
                out=dstT[:, sl * SLW:(sl + 1) * SLW],
                in0=pp[:, 0:SLW], scalar1=b_sb[:, t:t + 1])
            vclock[0] += mm_cost(KC * SLW, KC)

        def vproj(jt, pair=None):
            key = ('v', jt)
            if key in emitted:
                return
            emitted.add(key)
            sl, half = jt // 2, jt % 2
            x_t = x_slots[('v', sl)]
            pp = psum.tile([P, 512], f32, tag="work", bufs=2, name="pv")
            for c in range(KC):
                nc.tensor.matmul(
                    out=pp[:, 0:GD],
                    lhsT=x_t[:, c, half * P:(half + 1) * P],
                    rhs=wv_sb[:, c, :],
                    start=(c == 0), stop=(c == KC - 1))
            nc.vector.tensor_copy(
                out=v_sb[:, jt], in_=pp[:, 0:GD].rearrange("p (h e) -> p h e",
                                                           e=DK))
            vclock[0] += mm_cost(KC * GD, KC)

        probs_tiles = {}
        pvaccs = {}
        sc_seq = [0]

        def scores_exp(ih, pair, jt):
            qkproj('k', pair, jt // 2)
            for sl in range(ih * 4, ih * 4 + 4):
                qkproj('q', pair, sl)
            for hp in range(2):
                sc = psum.tile([P, 1024], f32, tag="sc", bufs=2,
                               name=f"sc{sc_seq[0] % 2}")
                sc_seq[0] += 1
                for iq in range(2):
                    nc.tensor.matmul(
                        out=sc[:, iq * 512:(iq + 1) * 512],
                        lhsT=kT_sb[pair][hp * DK:(hp + 1) * DK,
                                         jt * P:(jt + 1) * P],
                        rhs=qT_sb[pair][hp * DK:(hp + 1) * DK,
                                        ih * 1024 + iq * 512:
                                        ih * 1024 + (iq + 1) * 512],
                        start=True, stop=True)
                pr = probs_pool.tile([P, 1024], bf16, tag="probs", name="pr")
                nc.scalar.activation(out=pr, in_=sc, func=Act.Exp, scale=0.125)
                probs_tiles[(ih, pair, jt, hp)] = pr
                vclock[0] += mm_cost(1024, 2)
                if DEBUG and (ih, pair, jt, hp) == (0, 0, 0, 0):
                    nc.sync.dma_start(out=dbg['pr000'][:, :], in_=pr)

        # PV + denominators run as per-(hp,ic) accumulation chains, split at
        # jt=JSPLIT: part1 runs inside the pair's own late steps (stashed to
        # SBUF), part2 + combine after the pair's scores finish.  PSUM rule:
        # only one in-flight chain per bank, sequential chains are fine.
        JSPLIT = 12
        dstate = {}
        in_tail = [False]   # when True (ACT idle), split copies ACT/DVE
        stage_pool = ctx.enter_context(tc.tile_pool(name="stage", bufs=1))

        def _pair_state(ih, pair):
            key = (ih, pair)
            if key not in dstate:
                acc = [psum.tile([P, 8, DK], f32, tag="pvacc", bufs=2,
                                 name=f"pva{hp2}") for hp2 in range(2)]
                rec = rec_pool.tile([P, 16], f32, tag="rec", name="rec")
                stage = stage_pool.tile([P, 16, DK], f32, tag="stg",
                                        name="stage")
                d1 = stage_pool.tile([P, 16], f32, tag="d1", name="d1")
                dstate[key] = (acc, rec, stage, d1)
            return dstate[key]

        def part1_unit(ih, pair, hp, ic):
            acc, rec, stage, d1 = _pair_state(ih, pair)
            h = 2 * pair + hp
            idx = hp * 8 + ic
            den = psum.tile([P, 1], f32, tag="work", bufs=2, name="den")
            for jt in range(JSPLIT):
                vproj(jt, pair)
                pr = probs_tiles[(ih, pair, jt, hp)]
                nc.tensor.matmul(
                    out=acc[hp][:, ic],
                    lhsT=pr[:, ic * P:(ic + 1) * P],
                    rhs=v_sb[:, jt, h],
                    start=(jt == 0), stop=(jt == JSPLIT - 1))
                nc.tensor.matmul(
                    out=den,
                    lhsT=pr[:, ic * P:(ic + 1) * P],
                    rhs=ones_sb,
                    start=(jt == 0), stop=(jt == JSPLIT - 1))
            nc.vector.tensor_copy(out=stage[:, idx], in_=acc[hp][:, ic])
            nc.vector.tensor_copy(out=d1[:, idx:idx + 1], in_=den)
            vclock[0] += mm_cost(JSPLIT * (DK + 1), 2 * JSPLIT)

        def drain_unit(ih, pair, hp, ic):
            acc, rec, stage, d1 = _pair_state(ih, pair)
            h = 2 * pair + hp
            idx = hp * 8 + ic
            den = psum.tile([P, 1], f32, tag="work", bufs=2, name="den")
            for jt in range(JSPLIT, ST):
                vproj(jt, pair)
                pr = probs_tiles[(ih, pair, jt, hp)]
                nc.tensor.matmul(
                    out=acc[hp][:, ic],
                    lhsT=pr[:, ic * P:(ic + 1) * P],
                    rhs=v_sb[:, jt, h],
                    start=(jt == JSPLIT), stop=(jt == ST - 1))
                nc.tensor.matmul(
                    out=den,
                    lhsT=pr[:, ic * P:(ic + 1) * P],
                    rhs=ones_sb,
                    start=(jt == JSPLIT), stop=(jt == ST - 1))
            dsum = stage_pool.tile([P, 1], f32, tag="dsum", bufs=2,
                                   name="dsum")
            nc.vector.tensor_add(out=dsum, in0=den, in1=d1[:, idx:idx + 1])
            nc.vector.reciprocal(out=rec[:, idx:idx + 1], in_=dsum)
            vclock[0] += mm_cost((ST - JSPLIT) * (DK + 1), 2 * (ST - JSPLIT))

        def finish_ic(ih, pair, ic):
            # combine part1+part2, normalize + transpose one 128-column block
            acc, rec, stage, d1 = dstate[(ih, pair)]
            at = attn_pool.tile([P, P], bf16, tag="attn", name="at")
            for hp in range(2):
                idx = hp * 8 + ic
                r = rec[:, idx:idx + 1]
                tsum = stage_pool.tile([P, DK], f32, tag="tsum", bufs=2,
                                       name="tsum")
                nc.vector.tensor_add(out=tsum, in0=acc[hp][:, ic],
                                     in1=stage[:, idx])
                if in_tail[0] and hp == 1:
                    nc.scalar.mul(out=at[:, hp * DK:(hp + 1) * DK],
                                  in_=tsum, mul=r)
                else:
                    nc.vector.tensor_scalar_mul(
                        out=at[:, hp * DK:(hp + 1) * DK],
                        in0=tsum, scalar1=r)
            tp = psum.tile([P, P], bf16, tag="work", bufs=2, name="tp")
            nc.tensor.transpose(out=tp, in_=at, identity=ident_sb)
            col = (ih * 8 + ic) * P
            if in_tail[0] and ic % 2 == 1:
                nc.scalar.copy(out=attT[pair][:, col:col + P], in_=tp)
            else:
                nc.vector.tensor_copy(out=attT[pair][:, col:col + P], in_=tp)
            if in_tail[0]:
                outproj(ih * 8 + ic)
            vclock[0] += mm_cost(P, 1)

        def boundary(ih, pair):
            acc, rec, stage, d1 = dstate.pop((ih, pair))
            if DEBUG and (ih, pair) == (0, 0):
                nc.sync.dma_start(out=dbg['rec00'][:, :], in_=rec)
            for jt in range(ST):
                for hp in range(2):
                    del probs_tiles[(ih, pair, jt, hp)]

        def outproj(st):
            osb = outs_pool.tile([P, D], bf16, tag="osb", name="osb")
            for nb in range(2):
                # in the tail the sc ring is idle; using it keeps the work
                # ring free for the drain/transpose tiles
                po = psum.tile([P, 512], f32,
                               tag="sc" if in_tail[0] else "work",
                               bufs=2, name="po")
                for c in range(2):
                    nc.tensor.matmul(
                        out=po,
                        lhsT=attT[c][:, st * P:(st + 1) * P],
                        rhs=wo_sb[:, c, nb * 512:(nb + 1) * 512],
                        start=(c == 0), stop=(c == 1))
                if in_tail[0] and nb == 0:
                    nc.scalar.copy(out=osb[:, nb * 512:(nb + 1) * 512],
                                   in_=po)
                else:
                    nc.vector.tensor_copy(out=osb[:, nb * 512:(nb + 1) * 512],
                                          in_=po)
            nc.sync.dma_start(out=out[st * P:(st + 1) * P, :], in_=osb)
            vclock[0] += mm_cost(2 * 512, 4)

        # ---------------- schedule ----------------
        STEP_NS = 2080.0

        # fillers: (deadline_step, est_cost_ns, emit_fn), deadline-sorted
        U = 900.0   # est ns for one 8-chunk projection unit
        fillers = deque()
        for g in range(1, 8):
            fillers.append((2 * g - 1, U, lambda g=g: qkproj('k', 0, g)))
        # xq ring order: t1's early slices must follow t0's closely; steps
        # 1-2 have slack (scores only)
        fillers.append((1, U, lambda: qkproj('q', 1, 2)))
        fillers.append((2, U, lambda: qkproj('q', 1, 3)))
        fillers.append((30, U, lambda: qkproj('k', 1, 0)))
        for g in range(1, 8):
            fillers.append((30 + 2 * g, U, lambda g=g: qkproj('k', 1, g)))
        # xv ring (4 slots): slice v+4 arrives ~step 8 and reuses slot 0
        for jt in range(JSPLIT):
            fillers.append((KNOBS['vp_base'] + jt // 2, U,
                            lambda jt=jt: vproj(jt)))
        for jt in range(JSPLIT, ST):
            fillers.append((13 + (jt - JSPLIT), U, lambda jt=jt: vproj(jt)))
        # xq ring has 4 slots (no recycling pressure): t0's late slices are
        # due before ih1 of pair0 (step 16), t1's before pair1 (step 48)
        for sl in range(4, 8):
            fillers.append((KNOBS['q47_t0'] + sl, U,
                            lambda sl=sl: qkproj('q', 0, sl)))
            fillers.append((KNOBS['q47_t1'] + sl, U,
                            lambda sl=sl: qkproj('q', 1, sl)))
        fillers = deque(sorted(fillers, key=lambda x: x[0]))
        late_fillers = deque()   # outproj st0-7, enabled after ih0 done

        # prologue: project both t's for the early xq slices (ring order) in
        # the DMA-wait gaps; kproj interleaves as soon as its DMA lands
        qkproj('q', 0, 0)
        qkproj('q', 1, 0)
        qkproj('q', 0, 1)
        qkproj('q', 1, 1)
        qkproj('k', 0, 0)
        qkproj('q', 0, 2)
        qkproj('q', 0, 3)

        # pair-major: all of pair0's attention first, so pair1's projections
        # land in the second half where the PE has slack
        steps = [(ih, pair, jt) for pair in range(2) for ih in range(2)
                 for jt in range(ST)]
        drainq = deque()        # (enq_step, ih, pair, hp, ic)
        remaining = {}          # (ih,pair) -> remaining drain units

        pending_fin = deque()   # software-pipeline finish_ic one ic behind

        def emit_drain(si, budget):
            _, dih, dpair, hp, ic, part = drainq.popleft()
            if part == 'p1':
                part1_unit(dih, dpair, hp, ic)
                return budget - 380.0
            drain_unit(dih, dpair, hp, ic)
            budget -= 200.0
            if hp == 1:
                pending_fin.append((dih, dpair, ic))
                if len(pending_fin) > 1:
                    finish_ic(*pending_fin.popleft())
                    budget -= 150.0
            remaining[(dih, dpair)] -= 1
            if remaining[(dih, dpair)] == 0:
                while pending_fin:
                    finish_ic(*pending_fin.popleft())
                boundary(dih, dpair)
                if (dih, dpair) == (0, 1):
                    for st in range(8):
                        late_fillers.append(lambda st=st: outproj(st))
            return budget

        p1units = [(ic, hp) for ic in range(8) for hp in range(2)]
        for si, (ih, pair, jt) in enumerate(steps):
            scores_exp(ih, pair, jt)
            budget = KNOBS['step_budget']
            if (ih, pair) == (1, 1) and jt >= JSPLIT:
                # last pair: part1 inside its own steps to shrink the tail
                for ic, hp in p1units[4 * (jt - JSPLIT):4 * (jt - JSPLIT) + 4]:
                    part1_unit(ih, pair, hp, ic)
                budget -= 4 * 380.0
            if jt == ST - 1:
                remaining[(ih, pair)] = 16
                if (ih, pair) != (1, 1):
                    for ic, hp in p1units:
                        drainq.append((si, ih, pair, hp, ic, 'p1'))
                for ic in range(8):
                    for hp in range(2):
                        drainq.append((si, ih, pair, hp, ic, 'p2'))
            # 1. hard-overdue fillers (ring safety) run regardless of budget
            while fillers and fillers[0][0] <= si - KNOBS['forced_age']:
                _, c, fn = fillers.popleft()
                fn()
                budget -= c
            # 2. due fillers, budget-gated
            while fillers and fillers[0][0] <= si and budget > 0:
                _, c, fn = fillers.popleft()
                fn()
                budget -= c
            # 3. drains: budget-gated, but a minimum rate once aged
            pops = 0
            while drainq and drainq[0][0] < si and pops < KNOBS['drain_cap'] \
                    and (budget > 0 or
                         si - drainq[0][0] >= KNOBS['drain_age']):
                budget = emit_drain(si, budget)
                pops += 1
            # 4. output projections of the finished half
            if si >= KNOBS['late_start']:
                while late_fillers and budget > 0:
                    late_fillers.popleft()()
                    budget -= U

        # tail: drain the last pair; its boundary emits outproj per-ic
        in_tail[0] = True
        si = len(steps)
        while late_fillers:
            late_fillers.popleft()()
        while drainq:
            emit_drain(si, 0.0)
        if DEBUG:
            nc.sync.dma_start(out=dbg['qT0'][:, :], in_=qT_sb[0])
            nc.sync.dma_start(out=dbg['kT0'][:, :], in_=kT_sb[0])
            nc.sync.dma_start(
                out=dbg['v'][:, :],
                in_=v_sb.rearrange("p a b c -> p (a b c)"))
            nc.sync.dma_start(out=dbg['attT0'][:, :], in_=attT[0])
            nc.sync.dma_start(out=dbg['attT1'][:, :], in_=attT[1])

    nc.finalize()
    return nc


def _pack_x(Xb):
    # [S, D] f32 -> [128, NSL*KC*SLW] bf16, slices sl-major, (c, s) inside
    a = Xb.reshape(NSL, SLW, KC, P).transpose(3, 0, 2, 1)
    return np.ascontiguousarray(a).reshape(P, NSL * KC * SLW).astype(BF16)


def kernel(Q, K, V, Wq, bq, Wk, bk, Wv, bv, Wo, bo):
    from concourse.bass_utils import run_bass_kernel_spmd

    f32 = np.float32
    Q = np.asarray(Q, f32)
    K = np.asarray(K, f32)
    V = np.asarray(V, f32)
    Wq = np.asarray(Wq, f32)
    Wk = np.asarray(Wk, f32)
    Wv = np.asarray(Wv, f32)
    Wo = np.asarray(Wo, f32)
    bq = np.asarray(bq, f32)
    bk = np.asarray(bk, f32)
    bv = np.asarray(bv, f32)
    bo = np.asarray(bo, f32)

    xp = {}
    for b in range(B):
        xp[('q', b)] = _pack_x(Q[b])
        xp[('k', b)] = _pack_x(K[b])
        xp[('v', b)] = _pack_x(V[b])

    ident = np.eye(P, dtype=BF16)

    def pack_w(Wslice):
        # [1024, 256] -> [128, KC, 256] (p, c, m) -> flat
        a = Wslice.reshape(KC, P, GD).transpose(1, 0, 2)
        return np.ascontiguousarray(a).reshape(P, KC * GD).astype(BF16)

    in_maps = []
    for c in range(NCORES):
        b, g = c // GROUPS, c % GROUPS
        sl = slice(g * GD, (g + 1) * GD)
        wo_a = Wo[sl, :].reshape(2, P, D).transpose(1, 0, 2)
        in_maps.append({
            "xq": xp[('q', b)],
            "xk": xp[('k', b)],
            "xv": xp[('v', b)],
            "wq": pack_w(Wq[:, sl]),
            "wk": pack_w(Wk[:, sl]),
            "wv": pack_w(Wv[:, sl]),
            "wo": np.ascontiguousarray(wo_a).reshape(P, 2 * D).astype(BF16),
            "bq": np.ascontiguousarray(bq[sl].reshape(2, P).T),
            "bk": np.ascontiguousarray(bk[sl].reshape(2, P).T),
            "ident": ident,
        })

    if "nc" not in _cached:
        _cached["nc"] = _build_bass()
    nc = _cached["nc"]

    try:
        res = run_bass_kernel_spmd(nc, in_maps, core_ids=list(range(NCORES)))
    except ModuleNotFoundError:
        os.environ["BASS_NEVER_TRACE"] = "1"
        res = run_bass_kernel_spmd(nc, in_maps, core_ids=list(range(NCORES)))
    if res.exec_time_ns is not None:
        print(f"HW exec time: {res.exec_time_ns} ns")

    bo_eff = (bv @ Wo + bo).astype(f32)
    out = np.zeros((B, S, D), f32)
    for c in range(NCORES):
        b = c // GROUPS
        out[b] += res.results[c]["out"].astype(f32)
    out += bo_eff
    return out
